# revision 1
# baseline (speedup 1.0000x reference)
"""2D Haar DWT (LL subband) on 8 Trainium2 NeuronCores.

Reference computes LL = M0 @ x @ M1 per (n, c) image, where M0/M1 are the
Haar analysis low-pass matrices: every output element is the 2x2 box sum of
the input scaled by (1/sqrt(2))^2.  That makes the kernel a pure streaming
2x2-pool: memory-bound, no matmul needed.

Sharding: data-parallel over N (8 images of (32, 512, 512) -> one per core),
no communication.  Per core, the 32 channels are processed as 16 "units" of
2 channels: one fully contiguous 2 MiB input DMA per unit ([128, 4096] tile,
8 consecutive rows per partition, 16 KiB/partition descriptors), one 3D-AP
row-pair add + one stride-2 column-pair add on DVE, a c^2 scale on ACT, and
one contiguous 512 KiB output DMA ([128, 1024] tile, 4 output rows per
partition, 4 KiB/partition descriptors).

Raw Bass (no Tile): the SP sequencer issues input DMAs, ACT issues the
output DMAs right after its scale op.  All cross-engine deps are standalone
wait_ge instructions on the consuming sequencer, so no DMA descriptor ever
carries more than its completion increment (walrus DIRECT2D allows only one
embedded sync wait).

Measured (rep-slope harness, real TRN2): ~81 us/core steady-state vs a
~80 us/core loads-only bound (33.5 MB @ ~420 GB/s); stores are fully hidden
by the burst batching.  Remaining untested hypotheses, each worth <~2%:
32 KiB read descriptors (4-channel load units), a half-size first unit to
shorten the one-shot ramp, G=16 single-burst stores (needs B=2, likely
regresses the load pipeline).
"""

import time
from contextlib import ExitStack

import numpy as np

import concourse.bass as bass
import concourse.mybir as mybir
from concourse.bass_utils import run_bass_kernel_spmd

N, C, H, W = 8, 32, 512, 512
N_CORES = 8
# Match the reference's effective multiplier fl(c)*fl(c), c = f32(1/sqrt(2)).
_C = np.float32(1.0) / np.sqrt(np.float32(2.0))
C2 = float(np.float32(_C * _C))

_F32 = mybir.dt.float32


def build_nc(B: int = 8, n_img: int = C, h: int = H, w: int = W) -> bass.Bass:
    """B = pipeline depth (SBUF slots per stage).

    n_img/h reinterpret the (C, H, W) input as (n_img, h, W) row-units; with
    h=1024 each unit covers two channels in one contiguous DMA.
    """
    C_, H_, W = n_img, h, w  # noqa: N806 - shadow module constants for the body
    R = H_ // 256  # row-pair groups per partition (tile holds 2R rows/partition)
    assert H_ % 256 == 0 and R >= 1
    nc = bass.Bass()
    x = nc.dram_tensor("x", [C_, H_, W], _F32, kind="ExternalInput")
    y = nc.dram_tensor("y", [C_, H_ // 2, W // 2], _F32, kind="ExternalOutput")

    with ExitStack() as ctx:
        t = [
            ctx.enter_context(nc.sbuf_tensor(f"t{i}", [128, 2 * R * W], _F32))
            for i in range(B)
        ]
        v = [
            ctx.enter_context(nc.sbuf_tensor(f"v{i}", [128, R * W], _F32))
            for i in range(B)
        ]
        o = [
            ctx.enter_context(nc.sbuf_tensor(f"o{i}", [128, R * W // 2], _F32))
            for i in range(B)
        ]
        # Group buffers for burst stores: G units' scaled outputs accumulate in
        # one buffer, stored with a single grouped DMA.  Measured on HW: fine-
        # grained per-unit stores interleave with the load stream and cost
        # ~25 us/core in HBM read/write turnaround; batching stores into 4 MiB
        # bursts (2 per pass) hides them almost entirely (~135 -> ~81 us/core).
        G = min(8, C_)
        assert C_ % G == 0
        sg = [
            ctx.enter_context(nc.sbuf_tensor(f"sg{i}", [128, G * R * W // 2], _F32))
            for i in range(2)
        ]

        # DMA completions across dynamic queues are unordered, so a single
        # counting semaphore cannot identify WHICH transfer finished: use one
        # semaphore per buffer slot (only that slot's DMA bumps it).
        dma_in = [nc.alloc_semaphore(f"dma_in{i}") for i in range(B)]
        dma_out = [nc.alloc_semaphore(f"dma_out{i}") for i in range(2)]
        dve_sem = nc.alloc_semaphore("dve_sem")
        act_sem = nc.alloc_semaphore("act_sem")

        # no GPSIMD instructions anywhere in this kernel: skip its expensive
        # dge_drain in the exit barrier
        with nc.Block(no_gpsimd_drain=True) as block:

            @block.sync
            def _(sync):
                for c in range(C_):
                    if c >= B:
                        # WAR: vadd(c-B) must be done reading t-slot; this also
                        # transitively orders WAW vs in-DMA(c-B).
                        sync.wait_ge(dve_sem, 2 * (c - B) + 1)
                    sync.dma_start(
                        t[c % B][:], x[c].rearrange("(p r) w -> p (r w)", p=128)
                    ).then_inc(dma_in[c % B], 16)
                n_groups = C_ // G
                for k in range(2):
                    rounds = n_groups // 2 + (1 if k < n_groups % 2 else 0)
                    if rounds:
                        sync.wait_ge(dma_out[k], 16 * rounds)

            @block.vector
            def _(vector):
                for c in range(C_):
                    vector.wait_ge(dma_in[c % B], 16 * (c // B + 1))
                    if c >= B:
                        # WAR: hadd(c-B) must be done reading v-slot
                        vector.wait_ge(dve_sem, 2 * (c - B) + 2)
                    # vertical pair sums over each adjacent row pair
                    tt = t[c % B][:].rearrange("p (r q w) -> p r q w", r=R, q=2)
                    vector.tensor_add(
                        v[c % B][:].rearrange("p (r w) -> p r w", r=R),
                        tt[:, :, 0, :],
                        tt[:, :, 1, :],
                    ).then_inc(dve_sem)
                    # RAW: engines are pipelined, same-engine back-to-back needs sync
                    vector.wait_ge(dve_sem, 2 * c + 1)
                    if c >= B:
                        # WAR: scale(c-B) must be done reading o-slot
                        vector.wait_ge(act_sem, c - B + 1)
                    vv = v[c % B][:].rearrange("p (w two) -> p two w", two=2)
                    vector.tensor_add(o[c % B][:], vv[:, 0, :], vv[:, 1, :]).then_inc(
                        dve_sem
                    )

            @block.scalar
            def _(scalar):
                OW = R * W // 2  # output elems per unit per partition
                for c in range(C_):
                    g = c // G
                    k = g % 2
                    scalar.wait_ge(dve_sem, 2 * (c + 1))
                    if g >= 2:
                        # WAR: group-store(g-2) must be done reading this buffer
                        scalar.wait_ge(dma_out[k], 16 * (g // 2))
                    scalar.mul(
                        sg[k][:, (c % G) * OW : (c % G + 1) * OW], o[c % B][:], C2
                    ).then_inc(act_sem)
                    if c % G == G - 1:
                        # one grouped burst store for the whole G-unit window
                        a = g * G
                        ysub = y[a : a + G].rearrange("u (p r) w -> p u (r w)", p=128)
                        scalar.dma_start(
                            ysub, sg[k][:].rearrange("p (u rw) -> p u rw", u=G)
                        ).then_inc(dma_out[k], 16)._wait_ge(act_sem, c + 1)

    return nc


_NC_CACHE: bass.Bass | None = None

# The kernel program processes "units" of 1024 contiguous rows (2 channels at
# a time): one 2 MiB input DMA with 16 KiB/partition descriptors and one
# 512 KiB output DMA with 4 KiB/partition descriptors.  Fewer, larger DMAs
# than per-channel units at the same modeled time (descriptor-count halved).
_UNITS, _UH, _B = C // 2, 2 * H, 5


def run(x: np.ndarray, **spmd_kwargs):
    """x: (8, 32, 512, 512) f32 -> BassKernelResults over the 8 cores."""
    global _NC_CACHE
    if _NC_CACHE is None:
        _NC_CACHE = build_nc(B=_B, n_img=_UNITS, h=_UH)
    in_maps = [
        {"x": np.ascontiguousarray(x[n], dtype=np.float32).reshape(_UNITS, _UH, W)}
        for n in range(N_CORES)
    ]
    return run_bass_kernel_spmd(_NC_CACHE, in_maps, list(range(N_CORES)), **spmd_kwargs)


def kernel(**inputs: np.ndarray) -> np.ndarray:
    global _NC_CACHE
    x = np.asarray(inputs["input"], dtype=np.float32)
    last_err = None
    for attempt in range(3):
        try:
            res = run(x)
            return _out_full(res)
        except Exception as e:  # transient NRT/axon exec-unit flakes: rebuild + retry
            last_err = e
            _NC_CACHE = None
            time.sleep(10.0 + 20.0 * attempt)
    raise last_err


def _out_full(res) -> np.ndarray:
    return np.stack(
        [res.results[i]["y"].reshape(C, H // 2, W // 2) for i in range(N_CORES)], axis=0
    )



# revision 2
# speedup vs baseline: 1.0625x; 1.0625x over previous
"""2D Haar DWT (LL subband) on 8 Trainium2 NeuronCores.

Reference computes LL = M0 @ x @ M1 per (n, c) image, where M0/M1 are the
Haar analysis low-pass matrices: every output element is the 2x2 box sum of
the input scaled by (1/sqrt(2))^2.  That makes the kernel a pure streaming
2x2-pool: memory-bound, no matmul needed.

Sharding: data-parallel over N (8 images of (32, 512, 512) -> one per core),
no communication.  Per core, the 32 channels are processed as 16 "units" of
2 channels: one fully contiguous 2 MiB input DMA per unit ([128, 4096] tile,
8 consecutive rows per partition, 16 KiB/partition descriptors), one 3D-AP
row-pair add + one stride-2 column-pair add on DVE, a c^2 scale on ACT, and
one contiguous 512 KiB output DMA ([128, 1024] tile, 4 output rows per
partition, 4 KiB/partition descriptors).

Raw Bass (no Tile): the SP sequencer issues input DMAs, ACT issues the
output DMAs right after its scale op.  All cross-engine deps are standalone
wait_ge instructions on the consuming sequencer, so no DMA descriptor ever
carries more than its completion increment (walrus DIRECT2D allows only one
embedded sync wait).

Measured (rep-slope harness, real TRN2): ~81 us/core steady-state vs a
~80 us/core loads-only bound (33.5 MB @ ~420 GB/s); stores are fully hidden
by the burst batching.  Remaining untested hypotheses, each worth <~2%:
32 KiB read descriptors (4-channel load units), a half-size first unit to
shorten the one-shot ramp, G=16 single-burst stores (needs B=2, likely
regresses the load pipeline).
"""

import time
from contextlib import ExitStack

import numpy as np

import concourse.bass as bass
import concourse.mybir as mybir
from concourse.bass_utils import run_bass_kernel_spmd

N, C, H, W = 8, 32, 512, 512
N_CORES = 8
# Match the reference's effective multiplier fl(c)*fl(c), c = f32(1/sqrt(2)).
_C = np.float32(1.0) / np.sqrt(np.float32(2.0))
C2 = float(np.float32(_C * _C))

_F32 = mybir.dt.float32


def build_nc(B: int = 8, n_img: int = C, h: int = H, w: int = W) -> bass.Bass:
    """B = pipeline depth (SBUF slots per stage).

    n_img/h reinterpret the (C, H, W) input as (n_img, h, W) row-units; with
    h=1024 each unit covers two channels in one contiguous DMA.
    """
    C_, H_, W = n_img, h, w  # noqa: N806 - shadow module constants for the body
    R = H_ // 256  # row-pair groups per partition (tile holds 2R rows/partition)
    assert H_ % 256 == 0 and R >= 1
    nc = bass.Bass()
    x = nc.dram_tensor("x", [C_, H_, W], _F32, kind="ExternalInput")
    y = nc.dram_tensor("y", [C_, H_ // 2, W // 2], _F32, kind="ExternalOutput")

    with ExitStack() as ctx:
        t = [
            ctx.enter_context(nc.sbuf_tensor(f"t{i}", [128, 2 * R * W], _F32))
            for i in range(B)
        ]
        v = [
            ctx.enter_context(nc.sbuf_tensor(f"v{i}", [128, R * W], _F32))
            for i in range(B)
        ]
        o = [
            ctx.enter_context(nc.sbuf_tensor(f"o{i}", [128, R * W // 2], _F32))
            for i in range(B)
        ]
        # All units' scaled outputs accumulate in one SBUF buffer; stores are
        # scheduled so the serial DMA stream never idles: one big store for
        # units [0, C_-2) queues behind the remaining loads, then the last two
        # units store individually, keeping the final (non-overlappable)
        # transfer small.
        sg = ctx.enter_context(nc.sbuf_tensor("sg", [128, C_ * R * W // 2], _F32))
        store_splits = [(0, C_ - 2), (C_ - 2, C_ - 1), (C_ - 1, C_)]
        n_stores = len(store_splits)

        # DMA completions across dynamic queues are unordered, so a single
        # counting semaphore cannot identify WHICH transfer finished: use one
        # semaphore per buffer slot (only that slot's DMA bumps it).  The
        # output stores all land in distinct DRAM and are only awaited
        # together at the end, so one counting sem suffices for them.
        dma_in = [nc.alloc_semaphore(f"dma_in{i}") for i in range(B)]
        dma_out = nc.alloc_semaphore("dma_out")
        dve_sem = nc.alloc_semaphore("dve_sem")
        act_sem = nc.alloc_semaphore("act_sem")

        # no GPSIMD instructions anywhere in this kernel: skip its expensive
        # dge_drain in the exit barrier
        with nc.Block(no_gpsimd_drain=True) as block:

            @block.sync
            def _(sync):
                for c in range(C_):
                    if c >= B:
                        # WAR: vadd(c-B) must be done reading t-slot; this also
                        # transitively orders WAW vs in-DMA(c-B).
                        sync.wait_ge(dve_sem, 2 * (c - B) + 1)
                    sync.dma_start(
                        t[c % B][:], x[c].rearrange("(p r) w -> p (r w)", p=128)
                    ).then_inc(dma_in[c % B], 16)
                sync.wait_ge(dma_out, 16 * n_stores)

            @block.vector
            def _(vector):
                for c in range(C_):
                    vector.wait_ge(dma_in[c % B], 16 * (c // B + 1))
                    if c >= B:
                        # WAR: hadd(c-B) must be done reading v-slot
                        vector.wait_ge(dve_sem, 2 * (c - B) + 2)
                    # vertical pair sums over each adjacent row pair
                    tt = t[c % B][:].rearrange("p (r q w) -> p r q w", r=R, q=2)
                    vector.tensor_add(
                        v[c % B][:].rearrange("p (r w) -> p r w", r=R),
                        tt[:, :, 0, :],
                        tt[:, :, 1, :],
                    ).then_inc(dve_sem)
                    # RAW: engines are pipelined, same-engine back-to-back needs sync
                    vector.wait_ge(dve_sem, 2 * c + 1)
                    if c >= B:
                        # WAR: scale(c-B) must be done reading o-slot
                        vector.wait_ge(act_sem, c - B + 1)
                    vv = v[c % B][:].rearrange("p (w two) -> p two w", two=2)
                    vector.tensor_add(o[c % B][:], vv[:, 0, :], vv[:, 1, :]).then_inc(
                        dve_sem
                    )

            @block.scalar
            def _(scalar):
                OW = R * W // 2  # output elems per unit per partition
                for c in range(C_):
                    scalar.wait_ge(dve_sem, 2 * (c + 1))
                    scalar.mul(
                        sg[:, c * OW : (c + 1) * OW], o[c % B][:], C2
                    ).then_inc(act_sem)
                    for a, b in store_splits:
                        if b == c + 1:
                            ysub = y[a:b].rearrange("u (p r) w -> p u (r w)", p=128)
                            scalar.dma_start(
                                ysub,
                                sg[:, a * OW : b * OW].rearrange(
                                    "p (u rw) -> p u rw", u=b - a
                                ),
                            ).then_inc(dma_out, 16)._wait_ge(act_sem, c + 1)

    return nc


_NC_CACHE: bass.Bass | None = None

# The kernel program processes "units" of 1024 contiguous rows (2 channels at
# a time): one 2 MiB input DMA with 16 KiB/partition descriptors and one
# 512 KiB output DMA with 4 KiB/partition descriptors.  Fewer, larger DMAs
# than per-channel units at the same modeled time (descriptor-count halved).
_UNITS, _UH, _B = C // 2, 2 * H, 5


def run(x: np.ndarray, **spmd_kwargs):
    """x: (8, 32, 512, 512) f32 -> BassKernelResults over the 8 cores."""
    global _NC_CACHE
    if _NC_CACHE is None:
        _NC_CACHE = build_nc(B=_B, n_img=_UNITS, h=_UH)
    in_maps = [
        {"x": np.ascontiguousarray(x[n], dtype=np.float32).reshape(_UNITS, _UH, W)}
        for n in range(N_CORES)
    ]
    return run_bass_kernel_spmd(_NC_CACHE, in_maps, list(range(N_CORES)), **spmd_kwargs)


def kernel(**inputs: np.ndarray) -> np.ndarray:
    global _NC_CACHE
    x = np.asarray(inputs["input"], dtype=np.float32)
    last_err = None
    for attempt in range(3):
        try:
            res = run(x)
            return _out_full(res)
        except Exception as e:  # transient NRT/axon exec-unit flakes: rebuild + retry
            last_err = e
            _NC_CACHE = None
            time.sleep(10.0 + 20.0 * attempt)
    raise last_err


def _out_full(res) -> np.ndarray:
    return np.stack(
        [res.results[i]["y"].reshape(C, H // 2, W // 2) for i in range(N_CORES)], axis=0
    )



# revision 4
# speedup vs baseline: 1.0625x; 1.0000x over previous
"""2D Haar DWT (LL subband) on 8 Trainium2 NeuronCores.

Reference computes LL = M0 @ x @ M1 per (n, c) image, where M0/M1 are the
Haar analysis low-pass matrices: every output element is the 2x2 box sum of
the input scaled by (1/sqrt(2))^2.  That makes the kernel a pure streaming
2x2-pool: memory-bound, no matmul needed.

Sharding: data-parallel over N (8 images of (32, 512, 512) -> one per core),
no communication.  Per core, the 32 channels are processed as 16 "units" of
2 channels: one fully contiguous 2 MiB input DMA per unit ([128, 4096] tile,
8 consecutive rows per partition, 16 KiB/partition descriptors), one 3D-AP
row-pair add + one stride-2 column-pair add on DVE, a c^2 scale on ACT, and
one contiguous 512 KiB output DMA ([128, 1024] tile, 4 output rows per
partition, 4 KiB/partition descriptors).

Raw Bass (no Tile): the SP sequencer issues input DMAs, ACT issues the
output DMAs right after its scale op.  All cross-engine deps are standalone
wait_ge instructions on the consuming sequencer, so no DMA descriptor ever
carries more than its completion increment (walrus DIRECT2D allows only one
embedded sync wait).

Store schedule: all 16 units' outputs accumulate in one 64 KiB/partition
SBUF buffer and are stored as [0,7) + [7,14) bursts (issued mid-stream, so
they queue behind pending loads and hide under the load stream on real HW)
plus single-unit stores for units 14 and 15.  This keeps the serialized DMA
stream gap-free from the first load to a small final store, so total time =
preamble (~1.0 us) + first-DMA issue latency (~1.35 us) + 40 MiB of DMA
transfers + DMA-sem propagation + exit barrier.  Cost-model timeline:
120,090 ns/core (down from 127,546 baseline, whose last 4 MiB burst store
chained serially after the final unit's compute, idling the DMA stream for
~7.5 us).  Per the InstructionCostModel, DMA transfers are fully serialized
at 360 GB/s, so 116.5 us of transfer time is the hard floor; the remaining
~3.6 us is the fixed framework preamble, HWDGE/DGE first-issue latency, and
the 900 ns DMA-sem propagation on the final store.
"""

import time
from contextlib import ExitStack

import numpy as np

import concourse.bass as bass
import concourse.mybir as mybir
from concourse.bass_utils import run_bass_kernel_spmd

N, C, H, W = 8, 32, 512, 512
N_CORES = 8
# Match the reference's effective multiplier fl(c)*fl(c), c = f32(1/sqrt(2)).
_C = np.float32(1.0) / np.sqrt(np.float32(2.0))
C2 = float(np.float32(_C * _C))

_F32 = mybir.dt.float32


def build_nc(B: int = 8, n_img: int = C, h: int = H, w: int = W) -> bass.Bass:
    """B = pipeline depth (SBUF slots per stage).

    n_img/h reinterpret the (C, H, W) input as (n_img, h, W) row-units; with
    h=1024 each unit covers two channels in one contiguous DMA.
    """
    C_, H_, W = n_img, h, w  # noqa: N806 - shadow module constants for the body
    R = H_ // 256  # row-pair groups per partition (tile holds 2R rows/partition)
    assert H_ % 256 == 0 and R >= 1
    nc = bass.Bass()
    x = nc.dram_tensor("x", [C_, H_, W], _F32, kind="ExternalInput")
    y = nc.dram_tensor("y", [C_, H_ // 2, W // 2], _F32, kind="ExternalOutput")

    with ExitStack() as ctx:
        t = [
            ctx.enter_context(nc.sbuf_tensor(f"t{i}", [128, 2 * R * W], _F32))
            for i in range(B)
        ]
        v = [
            ctx.enter_context(nc.sbuf_tensor(f"v{i}", [128, R * W], _F32))
            for i in range(B)
        ]
        o = [
            ctx.enter_context(nc.sbuf_tensor(f"o{i}", [128, R * W // 2], _F32))
            for i in range(B)
        ]
        # All units' scaled outputs accumulate in one SBUF buffer; stores are
        # scheduled so the DMA stream never idles: two burst stores interleave
        # with the load stream (hidden under loads on real HW), then the last
        # two units store individually, keeping the final (non-overlappable)
        # transfer small so the kernel tail is just sem-prop + exit barrier.
        sg = ctx.enter_context(nc.sbuf_tensor("sg", [128, C_ * R * W // 2], _F32))
        mid = (C_ - 2) // 2
        store_splits = [(0, mid), (mid, C_ - 2), (C_ - 2, C_ - 1), (C_ - 1, C_)]
        n_stores = len(store_splits)

        # DMA completions across dynamic queues are unordered, so a single
        # counting semaphore cannot identify WHICH transfer finished: use one
        # semaphore per buffer slot (only that slot's DMA bumps it).  The
        # output stores all land in distinct DRAM and are only awaited
        # together at the end, so one counting sem suffices for them.
        dma_in = [nc.alloc_semaphore(f"dma_in{i}") for i in range(B)]
        dma_out = nc.alloc_semaphore("dma_out")
        dve_sem = nc.alloc_semaphore("dve_sem")
        act_sem = nc.alloc_semaphore("act_sem")

        # no GPSIMD instructions anywhere in this kernel: skip its expensive
        # dge_drain in the exit barrier
        with nc.Block(no_gpsimd_drain=True) as block:

            @block.sync
            def _(sync):
                for c in range(C_):
                    if c >= B:
                        # WAR: vadd(c-B) must be done reading t-slot; this also
                        # transitively orders WAW vs in-DMA(c-B).
                        sync.wait_ge(dve_sem, 2 * (c - B) + 1)
                    sync.dma_start(
                        t[c % B][:], x[c].rearrange("(p r) w -> p (r w)", p=128)
                    ).then_inc(dma_in[c % B], 16)
                sync.wait_ge(dma_out, 16 * n_stores)

            @block.vector
            def _(vector):
                for c in range(C_):
                    vector.wait_ge(dma_in[c % B], 16 * (c // B + 1))
                    if c >= B:
                        # WAR: hadd(c-B) must be done reading v-slot
                        vector.wait_ge(dve_sem, 2 * (c - B) + 2)
                    # vertical pair sums over each adjacent row pair
                    tt = t[c % B][:].rearrange("p (r q w) -> p r q w", r=R, q=2)
                    vector.tensor_add(
                        v[c % B][:].rearrange("p (r w) -> p r w", r=R),
                        tt[:, :, 0, :],
                        tt[:, :, 1, :],
                    ).then_inc(dve_sem)
                    # RAW: engines are pipelined, same-engine back-to-back needs sync
                    vector.wait_ge(dve_sem, 2 * c + 1)
                    if c >= B:
                        # WAR: scale(c-B) must be done reading o-slot
                        vector.wait_ge(act_sem, c - B + 1)
                    vv = v[c % B][:].rearrange("p (w two) -> p two w", two=2)
                    vector.tensor_add(o[c % B][:], vv[:, 0, :], vv[:, 1, :]).then_inc(
                        dve_sem
                    )

            @block.scalar
            def _(scalar):
                OW = R * W // 2  # output elems per unit per partition
                for c in range(C_):
                    scalar.wait_ge(dve_sem, 2 * (c + 1))
                    scalar.mul(
                        sg[:, c * OW : (c + 1) * OW], o[c % B][:], C2
                    ).then_inc(act_sem)
                    for a, b in store_splits:
                        if b == c + 1:
                            ysub = y[a:b].rearrange("u (p r) w -> p u (r w)", p=128)
                            scalar.dma_start(
                                ysub,
                                sg[:, a * OW : b * OW].rearrange(
                                    "p (u rw) -> p u rw", u=b - a
                                ),
                            ).then_inc(dma_out, 16)._wait_ge(act_sem, c + 1)

    return nc


_NC_CACHE: bass.Bass | None = None

# The kernel program processes "units" of 1024 contiguous rows (2 channels at
# a time): one 2 MiB input DMA with 16 KiB/partition descriptors and one
# 512 KiB output DMA with 4 KiB/partition descriptors.  Fewer, larger DMAs
# than per-channel units at the same modeled time (descriptor-count halved).
_UNITS, _UH, _B = C // 2, 2 * H, 5


def run(x: np.ndarray, **spmd_kwargs):
    """x: (8, 32, 512, 512) f32 -> BassKernelResults over the 8 cores."""
    global _NC_CACHE
    if _NC_CACHE is None:
        _NC_CACHE = build_nc(B=_B, n_img=_UNITS, h=_UH)
    in_maps = [
        {"x": np.ascontiguousarray(x[n], dtype=np.float32).reshape(_UNITS, _UH, W)}
        for n in range(N_CORES)
    ]
    return run_bass_kernel_spmd(_NC_CACHE, in_maps, list(range(N_CORES)), **spmd_kwargs)


def kernel(**inputs: np.ndarray) -> np.ndarray:
    global _NC_CACHE
    x = np.asarray(inputs["input"], dtype=np.float32)
    last_err = None
    for attempt in range(3):
        try:
            res = run(x)
            return _out_full(res)
        except Exception as e:  # transient NRT/axon exec-unit flakes: rebuild + retry
            last_err = e
            _NC_CACHE = None
            time.sleep(10.0 + 20.0 * attempt)
    raise last_err


def _out_full(res) -> np.ndarray:
    return np.stack(
        [res.results[i]["y"].reshape(C, H // 2, W // 2) for i in range(N_CORES)], axis=0
    )



# revision 5
# speedup vs baseline: 1.0656x; 1.0029x over previous
"""2D Haar DWT (LL subband) on 8 Trainium2 NeuronCores.

Reference computes LL = M0 @ x @ M1 per (n, c) image, where M0/M1 are the
Haar analysis low-pass matrices: every output element is the 2x2 box sum of
the input scaled by (1/sqrt(2))^2.  That makes the kernel a pure streaming
2x2-pool: memory-bound, no matmul needed.

Sharding: data-parallel over N (8 images of (32, 512, 512) -> one per core),
no communication.  Per core, the 32 channels are processed as 16 "units" of
2 channels: one fully contiguous 2 MiB input DMA per unit ([128, 4096] tile,
8 consecutive rows per partition, 16 KiB/partition descriptors), one 3D-AP
row-pair add + one stride-2 column-pair add on DVE, and a c^2 scale on ACT
into a shared output buffer.

Raw Bass (no Tile, no Block): per-engine instruction streams are emitted
directly into the root basic block (saves the Block entry/exit branches on
the critical SP path), with the Block-equivalent exit sequence (per-engine
drain + sem-only all-engine barrier, skipping GPSIMD's expensive dge_drain).
All cross-engine deps are standalone wait_ge instructions on the consuming
sequencer, so no DMA descriptor carries more than its completion increment
(walrus DIRECT2D allows only one embedded sync wait).

Store schedule: all 16 units' outputs accumulate in one 64 KiB/partition
SBUF buffer and are stored as [0,7) + [7,14) bursts (issued mid-stream, so
they queue behind pending loads and hide under the load stream on real HW)
plus single-unit stores for units 14 and 15.  This keeps the serialized DMA
stream gap-free from the first load to a small final store: per the
InstructionCostModel, DMA transfers are fully serialized at 360 GB/s, so
the 40 MiB of per-core traffic (32 in + 8 out) sets a hard ~116.5 us floor;
everything else (framework preamble, HWDGE+DGE first-issue latency, final
DMA-sem propagation, exit barrier) adds ~3.2 us of unavoidable ends.

The framework's const-AP memsets (f32 0/1, bf16 1, u8 127) are dead code
here -- nothing in this kernel references a const AP (scalar.mul uses a Copy
activation with immediate scale/bias) -- and they sit on the entry barrier's
critical path via the Pool engine.  build_nc skips emitting them.

Cost-model timeline: 119,737 ns/core (baseline: 127,546; gap-free DMA floor
with framework ends: ~119.7k).
"""

import time
from contextlib import ExitStack

import numpy as np

import concourse.bass as bass
import concourse.mybir as mybir
from concourse.bass_utils import run_bass_kernel_spmd

N, C, H, W = 8, 32, 512, 512
N_CORES = 8
# Match the reference's effective multiplier fl(c)*fl(c), c = f32(1/sqrt(2)).
_C = np.float32(1.0) / np.sqrt(np.float32(2.0))
C2 = float(np.float32(_C * _C))

_F32 = mybir.dt.float32


def _make_bass_no_const_memsets() -> bass.Bass:
    """Construct a Bass module without the const-AP init memsets.

    The Bass constructor unconditionally emits four GPSIMD memsets to fill
    its const-AP tensors; this kernel never reads a const AP, and the
    memsets dominate the entry-barrier critical path (~250 ns).  Suppress
    them for this construction only, restoring the method immediately.
    """
    orig = bass.BassGpSimd.memset
    bass.BassGpSimd.memset = lambda self, ap, value: None
    try:
        return bass.Bass()
    finally:
        bass.BassGpSimd.memset = orig


def build_nc(B: int = 5, n_img: int = C // 2, h: int = 2 * H, w: int = W) -> bass.Bass:
    """B = pipeline depth (SBUF slots per load/compute stage).

    n_img/h reinterpret the (C, H, W) input as (n_img, h, W) row-units; with
    h=1024 each unit covers two channels in one contiguous DMA.
    """
    C_, H_, W = n_img, h, w  # noqa: N806 - shadow module constants for the body
    R = H_ // 256  # row-pair groups per partition (tile holds 2R rows/partition)
    assert H_ % 256 == 0 and R >= 1
    nc = _make_bass_no_const_memsets()
    x = nc.dram_tensor("x", [C_, H_, W], _F32, kind="ExternalInput")
    y = nc.dram_tensor("y", [C_, H_ // 2, W // 2], _F32, kind="ExternalOutput")

    with ExitStack() as ctx:
        t = [
            ctx.enter_context(nc.sbuf_tensor(f"t{i}", [128, 2 * R * W], _F32))
            for i in range(B)
        ]
        v = [
            ctx.enter_context(nc.sbuf_tensor(f"v{i}", [128, R * W], _F32))
            for i in range(B)
        ]
        o = [
            ctx.enter_context(nc.sbuf_tensor(f"o{i}", [128, R * W // 2], _F32))
            for i in range(B)
        ]
        # All units' scaled outputs accumulate in one SBUF buffer; stores are
        # scheduled so the DMA stream never idles: two burst stores interleave
        # with the load stream (hidden under loads on real HW), then the last
        # two units store individually, keeping the final (non-overlappable)
        # transfer small so the kernel tail is just sem-prop + exit barrier.
        sg = ctx.enter_context(nc.sbuf_tensor("sg", [128, C_ * R * W // 2], _F32))
        mid = (C_ - 2) // 2
        store_splits = [(0, mid), (mid, C_ - 2), (C_ - 2, C_ - 1), (C_ - 1, C_)]
        n_stores = len(store_splits)

        # DMA completions across dynamic queues are unordered, so a single
        # counting semaphore cannot identify WHICH transfer finished: use one
        # semaphore per buffer slot (only that slot's DMA bumps it).  The
        # output stores all land in distinct DRAM and are only awaited
        # together at the end, so one counting sem suffices for them.
        dma_in = [nc.alloc_semaphore(f"dma_in{i}") for i in range(B)]
        dma_out = nc.alloc_semaphore("dma_out")
        dve_sem = nc.alloc_semaphore("dve_sem")
        act_sem = nc.alloc_semaphore("act_sem")

        sync, vector, scalar = nc.sync, nc.vector, nc.scalar

        # SP stream: input loads + final completion wait
        for c in range(C_):
            if c >= B:
                # WAR: vadd(c-B) must be done reading t-slot; this also
                # transitively orders WAW vs in-DMA(c-B).
                sync.wait_ge(dve_sem, 2 * (c - B) + 1)
            sync.dma_start(
                t[c % B][:], x[c].rearrange("(p r) w -> p (r w)", p=128)
            ).then_inc(dma_in[c % B], 16)
        sync.wait_ge(dma_out, 16 * n_stores)

        # DVE stream: vertical row-pair add, then horizontal column-pair add
        for c in range(C_):
            vector.wait_ge(dma_in[c % B], 16 * (c // B + 1))
            if c >= B:
                # WAR: hadd(c-B) must be done reading v-slot
                vector.wait_ge(dve_sem, 2 * (c - B) + 2)
            tt = t[c % B][:].rearrange("p (r q w) -> p r q w", r=R, q=2)
            vector.tensor_add(
                v[c % B][:].rearrange("p (r w) -> p r w", r=R),
                tt[:, :, 0, :],
                tt[:, :, 1, :],
            ).then_inc(dve_sem)
            # RAW: engines are pipelined, same-engine back-to-back needs sync
            vector.wait_ge(dve_sem, 2 * c + 1)
            if c >= B:
                # WAR: scale(c-B) must be done reading o-slot
                vector.wait_ge(act_sem, c - B + 1)
            vv = v[c % B][:].rearrange("p (w two) -> p two w", two=2)
            vector.tensor_add(o[c % B][:], vv[:, 0, :], vv[:, 1, :]).then_inc(dve_sem)

        # ACT stream: c^2 scale into the shared output buffer + burst stores
        OW = R * W // 2  # output elems per unit per partition
        for c in range(C_):
            scalar.wait_ge(dve_sem, 2 * (c + 1))
            scalar.mul(sg[:, c * OW : (c + 1) * OW], o[c % B][:], C2).then_inc(act_sem)
            for a, b in store_splits:
                if b == c + 1:
                    ysub = y[a:b].rearrange("u (p r) w -> p u (r w)", p=128)
                    scalar.dma_start(
                        ysub,
                        sg[:, a * OW : b * OW].rearrange("p (u rw) -> p u rw", u=b - a),
                    ).then_inc(dma_out, 16)._wait_ge(act_sem, c + 1)

        # Exit: drain non-GPSIMD engines + sem-only all-engine barrier (the
        # Block(no_gpsimd_drain=True) exit sequence, minus the body branches).
        gp = nc.gpsimd.engine
        for eng_type, eng in nc.engines.items():
            if eng_type == gp:
                continue
            d = mybir.InstDrain(
                name=nc.get_next_instruction_name(),
                ins=[],
                outs=[],
                bass_is_fusable=False,
            )
            d.engine = eng_type
            eng.add_instruction(d)
        nc.all_engine_barrier(sem_only=True)

    return nc


_NC_CACHE: bass.Bass | None = None

# The kernel program processes "units" of 1024 contiguous rows (2 channels at
# a time): one 2 MiB input DMA with 16 KiB/partition descriptors per unit and
# 4 KiB/partition descriptors on the burst stores.
_UNITS, _UH, _B = C // 2, 2 * H, 5


def run(x: np.ndarray, **spmd_kwargs):
    """x: (8, 32, 512, 512) f32 -> BassKernelResults over the 8 cores."""
    global _NC_CACHE
    if _NC_CACHE is None:
        _NC_CACHE = build_nc(B=_B, n_img=_UNITS, h=_UH)
    in_maps = [
        {"x": np.ascontiguousarray(x[n], dtype=np.float32).reshape(_UNITS, _UH, W)}
        for n in range(N_CORES)
    ]
    return run_bass_kernel_spmd(_NC_CACHE, in_maps, list(range(N_CORES)), **spmd_kwargs)


def kernel(**inputs: np.ndarray) -> np.ndarray:
    global _NC_CACHE
    x = np.asarray(inputs["input"], dtype=np.float32)
    last_err = None
    for attempt in range(3):
        try:
            res = run(x)
            return _out_full(res)
        except Exception as e:  # transient NRT/axon exec-unit flakes: rebuild + retry
            last_err = e
            _NC_CACHE = None
            time.sleep(10.0 + 20.0 * attempt)
    raise last_err


def _out_full(res) -> np.ndarray:
    return np.stack(
        [res.results[i]["y"].reshape(C, H // 2, W // 2) for i in range(N_CORES)], axis=0
    )


# revision 8
# speedup vs baseline: 1.0722x; 1.0062x over previous
"""2D Haar DWT (LL subband) on 8 Trainium2 NeuronCores.

Reference computes LL = M0 @ x @ M1 per (n, c) image, where M0/M1 are the
Haar analysis low-pass matrices: every output element is the 2x2 box sum of
the input scaled by (1/sqrt(2))^2.  That makes the kernel a pure streaming
2x2-pool: memory-bound, no matmul needed.

Sharding: data-parallel over N (8 images of (32, 512, 512) -> one per core),
no communication.  Per core, the 32 channels are processed as 16 "units" of
2 channels: one fully contiguous 2 MiB input DMA per unit ([128, 4096] tile,
8 consecutive rows per partition, 16 KiB/partition descriptors), one 3D-AP
row-pair add + one stride-2 column-pair add on DVE, and a c^2 scale on ACT
into a shared output buffer.

Raw Bass (no Tile, no Block): per-engine instruction streams are emitted
directly into the root basic block (saves the Block entry/exit branches on
the critical SP path), with the Block-equivalent exit sequence (per-engine
drain + sem-only all-engine barrier, skipping GPSIMD's expensive dge_drain).
All cross-engine deps are standalone wait_ge instructions on the consuming
sequencer, so no DMA descriptor carries more than its completion increment
(walrus DIRECT2D allows only one embedded sync wait).

Store schedule: all 16 units' outputs accumulate in one 64 KiB/partition
SBUF buffer and are stored as [0,7) + [7,14) bursts (issued mid-stream, so
they queue behind pending loads and hide under the load stream on real HW)
plus single-unit stores for units 14 and 15.  This keeps the serialized DMA
stream gap-free from the first load to a small final store: per the
InstructionCostModel, DMA transfers are fully serialized at 360 GB/s, so
the 40 MiB of per-core traffic (32 in + 8 out) sets a hard ~116.5 us floor;
everything else (framework preamble, HWDGE+DGE first-issue latency, final
DMA-sem propagation, exit barrier) adds ~3.2 us of unavoidable ends.

The framework's const-AP memsets (f32 0/1, bf16 1, u8 127) are dead code
here -- nothing in this kernel references a const AP (scalar.mul uses a Copy
activation with immediate scale/bias) -- and they sit on the entry barrier's
critical path via the Pool engine.  build_nc skips emitting them.

Cost-model timeline: 119,737 ns/core (baseline: 127,546; gap-free DMA floor
with framework ends: ~119.7k).
"""

import time
from contextlib import ExitStack

import numpy as np

import concourse.bass as bass
import concourse.mybir as mybir
from concourse.bass_utils import run_bass_kernel_spmd

N, C, H, W = 8, 32, 512, 512
N_CORES = 8
# Match the reference's effective multiplier fl(c)*fl(c), c = f32(1/sqrt(2)).
_C = np.float32(1.0) / np.sqrt(np.float32(2.0))
C2 = float(np.float32(_C * _C))

_F32 = mybir.dt.float32


def _make_bass_lean() -> bass.Bass:
    """Construct a Bass module without the const-AP init memsets and without
    the constructor's entry all-engine barrier.

    The Bass constructor unconditionally emits four GPSIMD memsets filling
    its const-AP tensors and a full all-engine barrier ordering every
    engine's register preamble before any user code.  Neither is needed by
    this kernel: nothing here reads a const AP (scalar.mul lowers to a Copy
    activation with immediate scale/bias), and all cross-engine ordering is
    semaphore-enforced (DVE waits on the load-DMA sem, ACT on DVE's sem;
    each engine's own register init precedes its own ops by program order;
    no engine reads another engine's registers).  Dropping them takes the
    entry-barrier wait (~730 ns) off the SP critical path that issues the
    first input DMA.  Both methods are restored immediately; only this one
    module is built without them.
    """
    orig_ms = bass.BassGpSimd.memset
    orig_b = bass.Bass.all_engine_barrier
    bass.BassGpSimd.memset = lambda self, ap, value: None
    bass.Bass.all_engine_barrier = lambda self, *a, **k: None
    try:
        return bass.Bass()
    finally:
        bass.BassGpSimd.memset = orig_ms
        bass.Bass.all_engine_barrier = orig_b


def build_nc(B: int = 5, n_img: int = C // 2, h: int = 2 * H, w: int = W) -> bass.Bass:
    """B = pipeline depth (SBUF slots per load/compute stage).

    n_img/h reinterpret the (C, H, W) input as (n_img, h, W) row-units; with
    h=1024 each unit covers two channels in one contiguous DMA.
    """
    C_, H_, W = n_img, h, w  # noqa: N806 - shadow module constants for the body
    R = H_ // 256  # row-pair groups per partition (tile holds 2R rows/partition)
    assert H_ % 256 == 0 and R >= 1
    nc = _make_bass_lean()
    x = nc.dram_tensor("x", [C_, H_, W], _F32, kind="ExternalInput")
    y = nc.dram_tensor("y", [C_, H_ // 2, W // 2], _F32, kind="ExternalOutput")

    with ExitStack() as ctx:
        t = [
            ctx.enter_context(nc.sbuf_tensor(f"t{i}", [128, 2 * R * W], _F32))
            for i in range(B)
        ]
        v = [
            ctx.enter_context(nc.sbuf_tensor(f"v{i}", [128, R * W], _F32))
            for i in range(B)
        ]
        o = [
            ctx.enter_context(nc.sbuf_tensor(f"o{i}", [128, R * W // 2], _F32))
            for i in range(B)
        ]
        # All units' scaled outputs accumulate in one SBUF buffer; stores are
        # scheduled so the DMA stream never idles: two burst stores interleave
        # with the load stream (hidden under loads on real HW), then the last
        # two units store individually, keeping the final (non-overlappable)
        # transfer small so the kernel tail is just sem-prop + exit barrier.
        sg = ctx.enter_context(nc.sbuf_tensor("sg", [128, C_ * R * W // 2], _F32))
        mid = (C_ - 2) // 2
        store_splits = [(0, mid), (mid, C_ - 2), (C_ - 2, C_ - 1), (C_ - 1, C_)]
        n_stores = len(store_splits)

        # DMA completions across dynamic queues are unordered, so a single
        # counting semaphore cannot identify WHICH transfer finished: use one
        # semaphore per buffer slot (only that slot's DMA bumps it).  The
        # output stores all land in distinct DRAM and are only awaited
        # together at the end, so one counting sem suffices for them.
        dma_in = [nc.alloc_semaphore(f"dma_in{i}") for i in range(B)]
        dma_out = nc.alloc_semaphore("dma_out")
        dve_sem = nc.alloc_semaphore("dve_sem")
        act_sem = nc.alloc_semaphore("act_sem")

        sync, vector, scalar = nc.sync, nc.vector, nc.scalar

        # SP stream: input loads + final completion wait
        for c in range(C_):
            if c >= B:
                # WAR: vadd(c-B) must be done reading t-slot; this also
                # transitively orders WAW vs in-DMA(c-B).
                sync.wait_ge(dve_sem, 2 * (c - B) + 1)
            sync.dma_start(
                t[c % B][:], x[c].rearrange("(p r) w -> p (r w)", p=128)
            ).then_inc(dma_in[c % B], 16)
        sync.wait_ge(dma_out, 16 * n_stores)

        # DVE stream: vertical row-pair add, then horizontal column-pair add
        for c in range(C_):
            vector.wait_ge(dma_in[c % B], 16 * (c // B + 1))
            if c >= B:
                # WAR: hadd(c-B) must be done reading v-slot
                vector.wait_ge(dve_sem, 2 * (c - B) + 2)
            tt = t[c % B][:].rearrange("p (r q w) -> p r q w", r=R, q=2)
            vector.tensor_add(
                v[c % B][:].rearrange("p (r w) -> p r w", r=R),
                tt[:, :, 0, :],
                tt[:, :, 1, :],
            ).then_inc(dve_sem)
            # RAW: engines are pipelined, same-engine back-to-back needs sync
            vector.wait_ge(dve_sem, 2 * c + 1)
            if c >= B:
                # WAR: scale(c-B) must be done reading o-slot
                vector.wait_ge(act_sem, c - B + 1)
            vv = v[c % B][:].rearrange("p (w two) -> p two w", two=2)
            vector.tensor_add(o[c % B][:], vv[:, 0, :], vv[:, 1, :]).then_inc(dve_sem)

        # ACT stream: c^2 scale into the shared output buffer + burst stores
        OW = R * W // 2  # output elems per unit per partition
        for c in range(C_):
            scalar.wait_ge(dve_sem, 2 * (c + 1))
            scalar.mul(sg[:, c * OW : (c + 1) * OW], o[c % B][:], C2).then_inc(act_sem)
            for a, b in store_splits:
                if b == c + 1:
                    ysub = y[a:b].rearrange("u (p r) w -> p u (r w)", p=128)
                    scalar.dma_start(
                        ysub,
                        sg[:, a * OW : b * OW].rearrange("p (u rw) -> p u rw", u=b - a),
                    ).then_inc(dma_out, 16)._wait_ge(act_sem, c + 1)

        # Exit: drain non-GPSIMD engines.  No exit barrier: every DMA is
        # sem-confirmed complete before SP passes its final wait (loads are
        # implied by the compute chain, stores by dma_out), every sem inc an
        # engine produces precedes its drain in program order, so each engine
        # can simply drain and halt.
        gp = nc.gpsimd.engine
        for eng_type, eng in nc.engines.items():
            if eng_type == gp:
                continue
            d = mybir.InstDrain(
                name=nc.get_next_instruction_name(),
                ins=[],
                outs=[],
                bass_is_fusable=False,
            )
            d.engine = eng_type
            eng.add_instruction(d)

    return nc


_NC_CACHE: bass.Bass | None = None

# The kernel program processes "units" of 1024 contiguous rows (2 channels at
# a time): one 2 MiB input DMA with 16 KiB/partition descriptors per unit and
# 4 KiB/partition descriptors on the burst stores.
_UNITS, _UH, _B = C // 2, 2 * H, 5


def run(x: np.ndarray, **spmd_kwargs):
    """x: (8, 32, 512, 512) f32 -> BassKernelResults over the 8 cores."""
    global _NC_CACHE
    if _NC_CACHE is None:
        _NC_CACHE = build_nc(B=_B, n_img=_UNITS, h=_UH)
    in_maps = [
        {"x": np.ascontiguousarray(x[n], dtype=np.float32).reshape(_UNITS, _UH, W)}
        for n in range(N_CORES)
    ]
    return run_bass_kernel_spmd(_NC_CACHE, in_maps, list(range(N_CORES)), **spmd_kwargs)


def kernel(**inputs: np.ndarray) -> np.ndarray:
    global _NC_CACHE
    x = np.asarray(inputs["input"], dtype=np.float32)
    last_err = None
    for attempt in range(3):
        try:
            res = run(x)
            return _out_full(res)
        except Exception as e:  # transient NRT/axon exec-unit flakes: rebuild + retry
            last_err = e
            _NC_CACHE = None
            time.sleep(10.0 + 20.0 * attempt)
    raise last_err


def _out_full(res) -> np.ndarray:
    return np.stack(
        [res.results[i]["y"].reshape(C, H // 2, W // 2) for i in range(N_CORES)], axis=0
    )
